# revision 39
# baseline (speedup 1.0000x reference)
"""BertLexer Trainium2 kernel.

Computes, for full inputs
    word_indices [16,256] int, span_start/span_end [16,256] int,
    W_embed [50002,256] f32, hidden_states [12,16,512,768] f32
the reference
    word_emb = W_embed[word_indices]                                # [B,W,E]
    bert_sub = hidden_states.mean(axis=0)                           # [B,S,H]
    bert_emb[b,w] = mean(bert_sub[b, span_start:span_end])          # [B,W,H]
    out = concat([word_emb, bert_emb], axis=2)                      # [B,W,E+H]

Strategy: data-parallel over the batch dim across 8 NeuronCores (2 batches
per core).  Only subwords below max(span_end) are ever referenced, so the
host slices hidden_states to SP = 384+T rows per batch before staging
(seed-0 inputs: max span_end = 400, T = 16; rebuilt for other inputs).

v5 structure (trace-driven).  Hard-won scheduling facts baked in:
- Tile list-schedules per engine but keeps emission order among ready
  instructions, and a dma_start *instruction* costs ~0.7us on the
  issuing engine and parks it when the HWDGE ring (~4 outstanding) is
  full.  So: NOTHING that waits on a semaphore may be emitted on the
  sync/scalar engines before their stream issues.  maskT PSUM->SBUF
  copies therefore run on the DVE (which also builds the masks), not
  on ACT (v4: ACT's copies gated scalar's first full-layer issue to
  ~23us, starving the stream).
- The index tensors are HOST-transposed to [128, BPC*WT] so each is
  one contiguous [128,16B] DMA at the head of the sync queue (masks
  build at ~9us; as 4B-scatter DMAs they landed at 13-23us).
- BOTH batches' packed tails ([128,96] per layer) issue at the very
  head of both HWDGE queues: tail chains + the DRAM bounce that
  unflattens them to [T,768] complete by ~25us, so the tail matmuls
  (which OPEN each PSUM accumulation zone, start=True) never gate the
  drain.  In v4, b1's tails sat behind b0's whole stream; the bounce
  landed at ~98us and the entire drain serialized behind it.
- The two HWDGE queues carry only the 29.5 MB hidden_states stream,
  roles swapped per batch so both drain together.  The last layer of
  each batch arrives as six [128,512/256] pieces -- one per (mask
  group j, PSUM chunk) -- whose DVE adds each unlock exactly the two
  fp32r matmuls consuming them: the post-stream drain is one 256-col
  add + two small matmuls + copies + stores.
- h_pool stays at 13 bufs: 14 shifts the ring onto SBUF banks that
  alias hsum and the [128,2304] DVE adds slow from 2.55us to 3.06us.
- W_embed is staged bf16 and the output is bf16 (host casts back to
  f32): 2e-3 relative rounding against the 2e-2 budget, ~1.3 MB/core
  less HBM traffic, half-size stores in the drain.
- gpsimd/SWDGE (~1.7us/op serial) carries only: iota, the tail-sum
  bounces (early), the embedding gathers (not needed until stores),
  and b0's stores (emitted last so its in-order stream never parks).
"""

import sys

import numpy as np

if "/opt/trn_rl_repo" not in sys.path:
    sys.path.insert(0, "/opt/trn_rl_repo")

import concourse.bacc as bacc
import concourse.bass as bass
import concourse.mybir as mybir
import concourse.tile as tile
from concourse.masks import make_identity

B, W, S, H, L, E, V = 16, 256, 512, 768, 12, 256, 50002
NCORES = 8
BPC = B // NCORES  # batches per core
P = 128
WT = W // P  # word-index tiles per batch
SFULL = 3 * P  # subwords covered by the full-region tiles (s = 3p + j)
CF = 3 * H  # full-region tile cols (2304)
NCHUNKS = [(0, 512), (512, 256)]  # PSUM-bank-sized pieces of H

F32 = mybir.dt.float32
BF16 = mybir.dt.bfloat16
I32 = mybir.dt.int32


def build_program(T):
    """T = tail subword count (power of two <= 128, or 0). SP = 384 + T."""
    SP = SFULL + T
    CT = (T * H) // P if T else 0  # packed tail cols (6T)
    SPM = SFULL + T  # mask columns
    nc = bacc.Bacc(
        "TRN2", target_bir_lowering=False, debug=False, num_devices=NCORES
    )
    # index tensors are host-transposed to [P, BPC*WT]:
    # column (b*WT + wt) holds word-tile wt of batch b.
    wi = nc.dram_tensor("word_indices", [P, BPC * WT], I32, kind="ExternalInput").ap()
    ss = nc.dram_tensor("span_start", [P, BPC * WT], I32, kind="ExternalInput").ap()
    se = nc.dram_tensor("span_end", [P, BPC * WT], I32, kind="ExternalInput").ap()
    emb = nc.dram_tensor("W_embed", [V, E], BF16, kind="ExternalInput").ap()
    hs = nc.dram_tensor("hidden_states", [L, BPC, SP * H], F32, kind="ExternalInput").ap()
    out = nc.dram_tensor("out", [BPC, W, E + H], BF16, kind="ExternalOutput").ap()
    tsc = (
        nc.dram_tensor("tail_scratch", [BPC, T * H], F32, kind="Internal").ap()
        if T
        else None
    )

    with tile.TileContext(nc) as tc:
        with (
            tc.tile_pool(name="const", bufs=1) as const_pool,
            tc.tile_pool(name="idx", bufs=2) as idx_pool,
            tc.tile_pool(name="mask", bufs=2) as mask_pool,
            tc.tile_pool(name="maskT", bufs=2) as maskT_pool,
            tc.tile_pool(name="hbuf", bufs=13) as h_pool,
            tc.tile_pool(name="cbuf", bufs=2) as c_pool,
            tc.tile_pool(name="htail", bufs=L if T <= 16 else 6) as ht_pool,
            tc.tile_pool(name="hsum", bufs=2) as hsum_pool,
            tc.tile_pool(name="tailb", bufs=2) as tail_pool,
            tc.tile_pool(name="obuf", bufs=4) as o_pool,
            tc.tile_pool(name="ptr", bufs=2, space="PSUM") as ptr_pool,
            tc.tile_pool(name="pout", bufs=1, space="PSUM") as pout_pool,
        ):
            identity = const_pool.tile([P, P], F32)
            make_identity(nc, identity)
            # iota column c holds the subword index mapped to mask column c:
            # cols j*128+p (j<3) -> 3p+j; cols 384.. -> 384..SP-1 (tail).
            iota_i = const_pool.tile([P, SPM], I32)
            nc.gpsimd.iota(
                iota_i[:, 0:SFULL], pattern=[[1, 3], [3, P]], base=0,
                channel_multiplier=0,
            )
            if T:
                nc.gpsimd.iota(
                    iota_i[:, SFULL:SPM], pattern=[[1, T]], base=SFULL,
                    channel_multiplier=0,
                )
            iota_f = const_pool.tile([P, SPM], F32)
            nc.gpsimd.tensor_copy(iota_f, iota_i)

            # --- index loads: one contiguous [128, 16B] DMA per tensor
            # at the head of the sync queue ---
            ss_i = idx_pool.tile([P, BPC * WT], I32, bufs=1)
            se_i = idx_pool.tile([P, BPC * WT], I32, bufs=1)
            wi_i = idx_pool.tile([P, BPC * WT], I32, bufs=1)
            nc.sync.dma_start(out=ss_i, in_=ss[:, :])
            nc.sync.dma_start(out=se_i, in_=se[:, :])
            nc.sync.dma_start(out=wi_i, in_=wi[:, :])
            ss_f = idx_pool.tile([P, BPC * WT], F32, bufs=1)
            se_f = idx_pool.tile([P, BPC * WT], F32, bufs=1)
            scale = idx_pool.tile([P, BPC * WT], F32, bufs=1)
            nc.vector.tensor_copy(ss_f, ss_i)
            nc.vector.tensor_copy(se_f, se_i)
            len_f = idx_pool.tile([P, BPC * WT], F32, bufs=1)
            nc.vector.tensor_tensor(len_f, se_f, ss_f, op=mybir.AluOpType.subtract)
            rlen = idx_pool.tile([P, BPC * WT], F32, bufs=1)
            nc.vector.reciprocal(rlen, len_f)
            nc.vector.tensor_scalar_mul(scale, rlen, 1.0 / L)

            # =========== phase 1: masks + transposes (maskT copies on
            # the DVE -- putting them on ACT blocks the scalar queue's
            # stream issues behind their semaphore waits) ===========
            maskT_fulls, maskT_tails = [], []
            for b in range(BPC):
                maskT_full = maskT_pool.tile([P, 3 * W], F32, tag="mtf")
                maskT_tail = None
                if T:
                    maskT_tail = maskT_pool.tile(
                        [T, W], F32, tag="mtt", name=f"mtt_{b}"
                    )
                for wt in range(WT):
                    c = b * WT + wt
                    m2 = mask_pool.tile([P, SPM], F32, tag="m2")
                    nc.vector.tensor_scalar(
                        m2,
                        iota_f,
                        scalar1=se_f[:, c : c + 1],
                        scalar2=scale[:, c : c + 1],
                        op0=mybir.AluOpType.is_lt,
                        op1=mybir.AluOpType.mult,
                    )
                    mM = mask_pool.tile([P, SPM], F32, tag="mM")
                    nc.vector.scalar_tensor_tensor(
                        mM,
                        iota_f,
                        ss_f[:, c : c + 1],
                        m2,
                        op0=mybir.AluOpType.is_ge,
                        op1=mybir.AluOpType.mult,
                    )
                    for j in range(3):
                        ptr = ptr_pool.tile([P, P], F32, space="PSUM", tag="ptr")
                        nc.tensor.transpose(
                            ptr, mM[:, j * P : (j + 1) * P], identity
                        )
                        col = (j * WT + wt) * P
                        nc.vector.tensor_copy(maskT_full[:, col : col + P], ptr)
                    if T:
                        ptrT = ptr_pool.tile([T, P], F32, space="PSUM", tag="ptrT")
                        nc.tensor.transpose(ptrT, mM[:, SFULL:SPM], identity)
                        nc.vector.tensor_copy(
                            maskT_tail[:, wt * P : (wt + 1) * P], ptrT
                        )
                maskT_fulls.append(maskT_full)
                maskT_tails.append(maskT_tail)

            # =========== phase 2: ALL hidden_states DMA issues on the
            # two HWDGE queues (roles swapped per batch): b0's first two
            # fulls lead (fast stream ramp), then BOTH batches' packed
            # tails (their sums, bounces and the zone-opening tail
            # matmuls finish early), then the remaining fulls and the
            # six last-layer (group j, PSUM chunk) pieces per batch. ===
            engs = [
                (nc.sync, nc.scalar),  # (evens, odds) for b0
                (nc.scalar, nc.sync),  # for b1
            ]
            hsums_b, h_bigs_b = [], []
            for b in range(BPC):
                hsum_t = hsum_pool.tile([P, CF + 32], F32, tag="hsum", name=f"hs_{b}")
                hsums_b.append(hsum_t[:, 0:CF])
                h_bigs_b.append([])

            def issue_full(b, l):
                evens, odds = engs[b]
                hb = h_pool.tile([P, CF + 32], F32, tag="h", name=f"h_{b}_{l}")
                big_eng = evens if l % 2 == 0 else odds
                big_eng.dma_start(
                    out=hb[:, 0:CF],
                    in_=hs[l, b, 0 : SFULL * H].rearrange("(p x) -> p x", p=P),
                )
                h_bigs_b[b].append(hb)

            issue_full(0, 0)
            issue_full(0, 1)
            # combined-batch packed tails: ONE [128, 2*CT] tile per layer
            # (partition p: batch 0's 96 tail values then batch 1's), 12
            # DMAs instead of 24 and a single 11-add chain that runs
            # entirely at the stream head.
            h_tails = []
            if T:
                for l in range(L):
                    htl = ht_pool.tile(
                        [P, 2 * CT], F32, tag="ht", name=f"ht_{l}"
                    )
                    tail_eng = nc.sync if l % 2 == 0 else nc.scalar
                    tail_eng.dma_start(
                        out=htl.rearrange("p (b x) -> p b x", b=BPC),
                        in_=hs[l, :, SFULL * H : SP * H].rearrange(
                            "b (p x) -> p b x", p=P
                        ),
                    )
                    h_tails.append(htl)
            NFULL = L - 1  # layers l0..l10 stream whole; only l11 as pieces
            c11_b = [None, None]
            for l in range(2, NFULL):
                issue_full(0, l)
            # b1's first two layers BEFORE b0's last-layer pieces: its
            # DVE chain can start ~3us earlier; b0's piece matmuls have
            # tens of us of slack before b1's drain needs PSUM.
            issue_full(1, 0)
            issue_full(1, 1)
            for b in range(BPC):
                evens, odds = engs[b]
                if b > 0:
                    for l in range(2, NFULL):
                        issue_full(b, l)
                # last layer as (j, n0) pieces; j0/j2 on `odds` (which
                # carried 5 fulls), j1 on `evens` (6 fulls).  (Putting
                # ALL pieces on `odds` to equalize bytes was tried and
                # regressed 8us: it skews the per-batch segment sizes
                # against the early-issued next-batch fulls.)
                c11 = {}
                l11_ap = hs[L - 1, b, 0 : SFULL * H].rearrange("(p x) -> p x", p=P)
                for j in range(3):
                    ch_eng = evens if j == 1 else odds
                    for n0, nl in NCHUNKS:
                        cb = c_pool.tile(
                            [P, nl], F32, tag=f"c{j}n{n0}", name=f"c_{b}_{j}_{n0}"
                        )
                        ch_eng.dma_start(
                            out=cb, in_=l11_ap[:, j * H + n0 : j * H + n0 + nl]
                        )
                        c11[(j, n0)] = cb
                c11_b[b] = c11

            # =========== phase 3a: the combined tail chain (all inputs
            # land at the stream head), both DRAM bounces, then the
            # embedding gathers (needed only by the stores) ===========
            tail16_b = [None, None]
            if T:
                hsumt = hsum_pool.tile([P, 2 * CT], F32, tag="hsumt")
                nc.vector.tensor_tensor(
                    hsumt, h_tails[0], h_tails[1], op=mybir.AluOpType.add
                )
                for l in range(2, L):
                    nc.vector.tensor_tensor(
                        hsumt, hsumt, h_tails[l], op=mybir.AluOpType.add
                    )
                for b in range(BPC):
                    nc.gpsimd.dma_start(
                        out=tsc[b, :].rearrange("(p x) -> p x", p=P),
                        in_=hsumt[:, b * CT : (b + 1) * CT],
                    )
                    tail16 = tail_pool.tile(
                        [T, H], F32, tag="t16", name=f"t16_{b}"
                    )
                    nc.gpsimd.dma_start(
                        out=tail16, in_=tsc[b, :].rearrange("(t x) -> t x", t=T)
                    )
                    tail16_b[b] = tail16
            obufs_b = []
            for b in range(BPC):
                obufs = []
                for wt in range(WT):
                    obuf = o_pool.tile(
                        [P, E + H], BF16, tag="obuf", name=f"obuf_{b}_{wt}"
                    )
                    nc.gpsimd.indirect_dma_start(
                        out=obuf[:, 0:E],
                        out_offset=None,
                        in_=emb[:, :],
                        in_offset=bass.IndirectOffsetOnAxis(
                            ap=wi_i[:, b * WT + wt : b * WT + wt + 1], axis=0
                        ),
                    )
                    obufs.append(obuf)
                obufs_b.append(obufs)
            # the LAST batch's word-embedding columns are final as soon
            # as the gathers land (~30us): store them now on the SWDGE
            # queue, so the end-of-kernel stores cover only the bert
            # columns (the very last transfer shrinks to 64 KB).
            for wt in range(WT):
                nc.gpsimd.dma_start(
                    out=out[BPC - 1, wt * P : (wt + 1) * P, 0:E],
                    in_=obufs_b[BPC - 1][wt][:, 0:E],
                )

            # =========== phase 3b: per-batch layer sums + span matmuls =
            b0_stores = []
            for b in range(BPC):
                h_bigs, c11, hsum = h_bigs_b[b], c11_b[b], hsums_b[b]
                maskT_full, maskT_tail = maskT_fulls[b], maskT_tails[b]
                tail16, obufs = tail16_b[b], obufs_b[b]
                # exact f32 sum of layers l0..l10 on the DVE
                nc.vector.tensor_tensor(
                    hsum, h_bigs[0][:, 0:CF], h_bigs[1][:, 0:CF],
                    op=mybir.AluOpType.add,
                )
                for l in range(2, NFULL):
                    nc.vector.tensor_tensor(
                        hsum, hsum, h_bigs[l][:, 0:CF],
                        op=mybir.AluOpType.add,
                    )
                # span matmuls: the tail matmul OPENS each PSUM zone
                # (its data has been ready since ~25us); then per (j, n0)
                # piece: the DVE add finalizes hsum[:, jH+n0 : jH+n0+nl]
                # and unlocks exactly the two matmuls that consume it.
                pouts = []
                for wt in range(WT):
                    pout = pout_pool.tile(
                        [P, H], F32, space="PSUM", tag=f"pout{wt}",
                        name=f"pout{wt}_{b}",
                    )
                    pouts.append(pout)
                if T:
                    for wt in range(WT):
                        for n0, nl in NCHUNKS:
                            nc.tensor.matmul(
                                pouts[wt][:, n0 : n0 + nl],
                                lhsT=maskT_tail[:, wt * P : (wt + 1) * P],
                                rhs=tail16[:, n0 : n0 + nl],
                                start=True,
                                stop=False,
                            )
                for j in range(3):
                    for n0, nl in NCHUNKS:
                        nc.vector.tensor_tensor(
                            hsum[:, j * H + n0 : j * H + n0 + nl],
                            hsum[:, j * H + n0 : j * H + n0 + nl],
                            c11[(j, n0)],
                            op=mybir.AluOpType.add,
                        )
                    # wt-major: consecutive matmuls share lhsT (one
                    # weight load can serve both PSUM chunks), and wt0's
                    # zone-stop fires earlier for the chasing copies.
                    for wt in range(WT):
                        col = (j * WT + wt) * P
                        for n0, nl in NCHUNKS:
                            nc.tensor.matmul(
                                pouts[wt][:, n0 : n0 + nl],
                                lhsT=maskT_full[:, col : col + P],
                                rhs=hsum[:, j * H + n0 : j * H + n0 + nl],
                                start=(j == 0 and not T),
                                stop=(j == 2),
                            )
                # PSUM -> bf16 row tiles.  Last batch: wt0 on ACT in
                # parallel with wt1 on the (by then idle) DVE, copies
                # AND bert-column stores split per PSUM chunk so each
                # chases its own j2-stop matmul (the word columns were
                # stored mid-stream); earlier batches both on ACT (a
                # DVE copy would stall the in-order DVE stream on this
                # batch's matmuls) with whole stores on SWDGE.
                if b < BPC - 1:
                    for wt in range(WT):
                        for n0, nl in NCHUNKS:
                            nc.scalar.copy(
                                obufs[wt][:, E + n0 : E + n0 + nl],
                                pouts[wt][:, n0 : n0 + nl],
                            )
                    # stores ride SWDGE but are EMITTED after the whole
                    # b1 block so the gpsimd engine's in-order stream
                    # isn't parked waiting on b0's obufs.
                    b0_stores.append((b, obufs))
                else:
                    for n0, nl in NCHUNKS:
                        nc.scalar.copy(
                            obufs[0][:, E + n0 : E + n0 + nl],
                            pouts[0][:, n0 : n0 + nl],
                        )
                        nc.scalar.dma_start(
                            out=out[b, 0:P, E + n0 : E + n0 + nl],
                            in_=obufs[0][:, E + n0 : E + n0 + nl],
                        )
                        nc.vector.tensor_copy(
                            obufs[1][:, E + n0 : E + n0 + nl],
                            pouts[1][:, n0 : n0 + nl],
                        )
                        nc.sync.dma_start(
                            out=out[b, P : 2 * P, E + n0 : E + n0 + nl],
                            in_=obufs[1][:, E + n0 : E + n0 + nl],
                        )
            for b, obufs in b0_stores:
                for wt in range(WT):
                    nc.gpsimd.dma_start(
                        out=out[b, wt * P : (wt + 1) * P, :], in_=obufs[wt]
                    )

    nc.compile()
    return nc


_NC = {}


def _tail_for(s_used):
    """Round the needed tail (beyond 384) up to a power of two <= 128."""
    if s_used <= SFULL:
        return 0
    t = s_used - SFULL
    p = 1
    while p < t:
        p *= 2
    return min(p, P)


def _get_program(T=16):
    if T not in _NC:
        _NC[T] = build_program(T)
    return _NC[T]


def _idx_stage(a):
    """[BPC, W] int -> [P, BPC*WT] with col (b*WT+wt) = word-tile wt of b."""
    a = np.asarray(a).astype(np.int32).reshape(BPC, WT, P)
    return np.ascontiguousarray(a.transpose(2, 0, 1).reshape(P, BPC * WT))


def make_in_maps(word_indices, span_start, span_end, W_embed, hidden_states, T):
    import ml_dtypes

    SP = SFULL + T
    emb = np.ascontiguousarray(np.asarray(W_embed).astype(ml_dtypes.bfloat16))
    in_maps = []
    for c in range(NCORES):
        bsl = slice(BPC * c, BPC * (c + 1))
        hsc = np.ascontiguousarray(
            hidden_states[:, bsl, :SP, :], dtype=np.float32
        ).reshape(L, BPC, SP * H)
        in_maps.append(
            {
                "word_indices": _idx_stage(word_indices[bsl]),
                "span_start": _idx_stage(span_start[bsl]),
                "span_end": _idx_stage(span_end[bsl]),
                "W_embed": emb,
                "hidden_states": hsc,
            }
        )
    return in_maps


def run(word_indices, span_start, span_end, W_embed, hidden_states, **run_kwargs):
    from concourse.bass_utils import run_bass_kernel_spmd

    s_used = int(np.max(np.asarray(span_end)[:, -1]))
    T = _tail_for(s_used)
    nc = _get_program(T)
    in_maps = make_in_maps(
        word_indices, span_start, span_end, W_embed, hidden_states, T
    )
    res = run_bass_kernel_spmd(nc, in_maps, core_ids=list(range(NCORES)), **run_kwargs)
    out = np.concatenate(
        [np.asarray(res.results[c]["out"], dtype=np.float32) for c in range(NCORES)],
        axis=0,
    )
    return out, res


def kernel(word_indices, span_start, span_end, W_embed, hidden_states):
    out, _ = run(word_indices, span_start, span_end, W_embed, hidden_states)
    return out


# revision 42
# speedup vs baseline: 1.0382x; 1.0382x over previous
"""BertLexer Trainium2 kernel.

Computes, for full inputs
    word_indices [16,256] int, span_start/span_end [16,256] int,
    W_embed [50002,256] f32, hidden_states [12,16,512,768] f32
the reference
    word_emb = W_embed[word_indices]                                # [B,W,E]
    bert_sub = hidden_states.mean(axis=0)                           # [B,S,H]
    bert_emb[b,w] = mean(bert_sub[b, span_start:span_end])          # [B,W,H]
    out = concat([word_emb, bert_emb], axis=2)                      # [B,W,E+H]

Strategy: data-parallel over the batch dim across 8 NeuronCores (2 batches
per core).  Only subwords below max(span_end) are ever referenced, so the
host slices hidden_states to SP = 384+T rows per batch before staging
(seed-0 inputs: max span_end = 400, T = 16; rebuilt for other inputs).

v5 structure (trace-driven).  Hard-won scheduling facts baked in:
- Tile list-schedules per engine but keeps emission order among ready
  instructions, and a dma_start *instruction* costs ~0.7us on the
  issuing engine and parks it when the HWDGE ring (~4 outstanding) is
  full.  So: NOTHING that waits on a semaphore may be emitted on the
  sync/scalar engines before their stream issues.  maskT PSUM->SBUF
  copies therefore run on the DVE (which also builds the masks), not
  on ACT (v4: ACT's copies gated scalar's first full-layer issue to
  ~23us, starving the stream).
- The index tensors are HOST-transposed to [128, BPC*WT] so each is
  one contiguous [128,16B] DMA at the head of the sync queue (masks
  build at ~9us; as 4B-scatter DMAs they landed at 13-23us).
- BOTH batches' packed tails ([128,96] per layer) issue at the very
  head of both HWDGE queues: tail chains + the DRAM bounce that
  unflattens them to [T,768] complete by ~25us, so the tail matmuls
  (which OPEN each PSUM accumulation zone, start=True) never gate the
  drain.  In v4, b1's tails sat behind b0's whole stream; the bounce
  landed at ~98us and the entire drain serialized behind it.
- The two HWDGE queues carry only the 29.5 MB hidden_states stream,
  roles swapped per batch so both drain together.  The last layer of
  each batch arrives as six [128,512/256] pieces -- one per (mask
  group j, PSUM chunk) -- whose DVE adds each unlock exactly the two
  fp32r matmuls consuming them: the post-stream drain is one 256-col
  add + two small matmuls + copies + stores.
- h_pool stays at 13 bufs: 14 shifts the ring onto SBUF banks that
  alias hsum and the [128,2304] DVE adds slow from 2.55us to 3.06us.
- W_embed is staged bf16 and the output is bf16 (host casts back to
  f32): 2e-3 relative rounding against the 2e-2 budget, ~1.3 MB/core
  less HBM traffic, half-size stores in the drain.
- gpsimd/SWDGE (~1.7us/op serial) carries only: iota, the tail-sum
  bounces (early), the embedding gathers (not needed until stores),
  and b0's stores (emitted last so its in-order stream never parks).
"""

import sys

import numpy as np

if "/opt/trn_rl_repo" not in sys.path:
    sys.path.insert(0, "/opt/trn_rl_repo")

import concourse.bacc as bacc
import concourse.bass as bass
import concourse.mybir as mybir
import concourse.tile as tile
from concourse.masks import make_identity

B, W, S, H, L, E, V = 16, 256, 512, 768, 12, 256, 50002
NCORES = 8
BPC = B // NCORES  # batches per core
P = 128
WT = W // P  # word-index tiles per batch
SFULL = 3 * P  # subwords covered by the full-region tiles (s = 3p + j)
CF = 3 * H  # full-region tile cols (2304)
NCHUNKS = [(0, 512), (512, 256)]  # PSUM-bank-sized pieces of H

F32 = mybir.dt.float32
BF16 = mybir.dt.bfloat16
I32 = mybir.dt.int32


def build_program(T):
    """T = tail subword count (power of two <= 128, or 0). SP = 384 + T."""
    SP = SFULL + T
    CT = (T * H) // P if T else 0  # packed tail cols (6T)
    SPM = SFULL + T  # mask columns
    nc = bacc.Bacc(
        "TRN2", target_bir_lowering=False, debug=False, num_devices=NCORES
    )
    # index tensors are host-transposed to [P, BPC*WT]:
    # column (b*WT + wt) holds word-tile wt of batch b.
    wi = nc.dram_tensor("word_indices", [P, BPC * WT], I32, kind="ExternalInput").ap()
    ss = nc.dram_tensor("span_start", [P, BPC * WT], I32, kind="ExternalInput").ap()
    se = nc.dram_tensor("span_end", [P, BPC * WT], I32, kind="ExternalInput").ap()
    emb = nc.dram_tensor("W_embed", [V, E], BF16, kind="ExternalInput").ap()
    hs = nc.dram_tensor("hidden_states", [L, BPC, SP * H], F32, kind="ExternalInput").ap()
    out = nc.dram_tensor("out", [BPC, W, E + H], BF16, kind="ExternalOutput").ap()
    tsc = (
        nc.dram_tensor("tail_scratch", [BPC, T * H], F32, kind="Internal").ap()
        if T
        else None
    )

    with tile.TileContext(nc) as tc:
        with (
            tc.tile_pool(name="const", bufs=1) as const_pool,
            tc.tile_pool(name="idx", bufs=2) as idx_pool,
            tc.tile_pool(name="mask", bufs=2) as mask_pool,
            tc.tile_pool(name="maskT", bufs=2) as maskT_pool,
            tc.tile_pool(name="hbuf", bufs=13) as h_pool,
            tc.tile_pool(name="cbuf", bufs=2) as c_pool,
            tc.tile_pool(name="htail", bufs=L if T <= 16 else 6) as ht_pool,
            tc.tile_pool(name="hsum", bufs=2) as hsum_pool,
            tc.tile_pool(name="tailb", bufs=2) as tail_pool,
            tc.tile_pool(name="obuf", bufs=4) as o_pool,
            tc.tile_pool(name="ptr", bufs=2, space="PSUM") as ptr_pool,
            tc.tile_pool(name="pout", bufs=1, space="PSUM") as pout_pool,
        ):
            identity = const_pool.tile([P, P], F32)
            make_identity(nc, identity)
            # iota column c holds the subword index mapped to mask column c:
            # cols j*128+p (j<3) -> 3p+j; cols 384.. -> 384..SP-1 (tail).
            iota_i = const_pool.tile([P, SPM], I32)
            nc.gpsimd.iota(
                iota_i[:, 0:SFULL], pattern=[[1, 3], [3, P]], base=0,
                channel_multiplier=0,
            )
            if T:
                nc.gpsimd.iota(
                    iota_i[:, SFULL:SPM], pattern=[[1, T]], base=SFULL,
                    channel_multiplier=0,
                )
            iota_f = const_pool.tile([P, SPM], F32)
            nc.gpsimd.tensor_copy(iota_f, iota_i)

            # --- index loads: one contiguous [128, 16B] DMA per tensor
            # at the head of the sync queue ---
            ss_i = idx_pool.tile([P, BPC * WT], I32, bufs=1)
            se_i = idx_pool.tile([P, BPC * WT], I32, bufs=1)
            wi_i = idx_pool.tile([P, BPC * WT], I32, bufs=1)
            nc.sync.dma_start(out=ss_i, in_=ss[:, :])
            nc.sync.dma_start(out=se_i, in_=se[:, :])
            nc.sync.dma_start(out=wi_i, in_=wi[:, :])
            ss_f = idx_pool.tile([P, BPC * WT], F32, bufs=1)
            se_f = idx_pool.tile([P, BPC * WT], F32, bufs=1)
            scale = idx_pool.tile([P, BPC * WT], F32, bufs=1)
            nc.vector.tensor_copy(ss_f, ss_i)
            nc.vector.tensor_copy(se_f, se_i)
            len_f = idx_pool.tile([P, BPC * WT], F32, bufs=1)
            nc.vector.tensor_tensor(len_f, se_f, ss_f, op=mybir.AluOpType.subtract)
            rlen = idx_pool.tile([P, BPC * WT], F32, bufs=1)
            nc.vector.reciprocal(rlen, len_f)
            nc.vector.tensor_scalar_mul(scale, rlen, 1.0 / L)

            # =========== phase 1: masks + transposes (maskT copies on
            # the DVE -- putting them on ACT blocks the scalar queue's
            # stream issues behind their semaphore waits) ===========
            maskT_fulls, maskT_tails = [], []
            for b in range(BPC):
                maskT_full = maskT_pool.tile([P, 3 * W], F32, tag="mtf")
                maskT_tail = None
                if T:
                    maskT_tail = maskT_pool.tile(
                        [T, W], F32, tag="mtt", name=f"mtt_{b}"
                    )
                for wt in range(WT):
                    c = b * WT + wt
                    m2 = mask_pool.tile([P, SPM], F32, tag="m2")
                    nc.vector.tensor_scalar(
                        m2,
                        iota_f,
                        scalar1=se_f[:, c : c + 1],
                        scalar2=scale[:, c : c + 1],
                        op0=mybir.AluOpType.is_lt,
                        op1=mybir.AluOpType.mult,
                    )
                    mM = mask_pool.tile([P, SPM], F32, tag="mM")
                    nc.vector.scalar_tensor_tensor(
                        mM,
                        iota_f,
                        ss_f[:, c : c + 1],
                        m2,
                        op0=mybir.AluOpType.is_ge,
                        op1=mybir.AluOpType.mult,
                    )
                    for j in range(3):
                        ptr = ptr_pool.tile([P, P], F32, space="PSUM", tag="ptr")
                        nc.tensor.transpose(
                            ptr, mM[:, j * P : (j + 1) * P], identity
                        )
                        col = (j * WT + wt) * P
                        nc.vector.tensor_copy(maskT_full[:, col : col + P], ptr)
                    if T:
                        ptrT = ptr_pool.tile([T, P], F32, space="PSUM", tag="ptrT")
                        nc.tensor.transpose(ptrT, mM[:, SFULL:SPM], identity)
                        nc.vector.tensor_copy(
                            maskT_tail[:, wt * P : (wt + 1) * P], ptrT
                        )
                maskT_fulls.append(maskT_full)
                maskT_tails.append(maskT_tail)

            # =========== phase 2: ALL hidden_states DMA issues on the
            # two HWDGE queues (roles swapped per batch): b0's first two
            # fulls lead (fast stream ramp), then BOTH batches' packed
            # tails (their sums, bounces and the zone-opening tail
            # matmuls finish early), then the remaining fulls and the
            # six last-layer (group j, PSUM chunk) pieces per batch. ===
            engs = [
                (nc.sync, nc.scalar),  # (evens, odds) for b0
                (nc.scalar, nc.sync),  # for b1
            ]
            hsums_b, h_bigs_b = [], []
            for b in range(BPC):
                hsum_t = hsum_pool.tile([P, CF + 32], F32, tag="hsum", name=f"hs_{b}")
                hsums_b.append(hsum_t[:, 0:CF])
                h_bigs_b.append([])

            def issue_full(b, l):
                evens, odds = engs[b]
                hb = h_pool.tile([P, CF + 32], F32, tag="h", name=f"h_{b}_{l}")
                big_eng = evens if l % 2 == 0 else odds
                big_eng.dma_start(
                    out=hb[:, 0:CF],
                    in_=hs[l, b, 0 : SFULL * H].rearrange("(p x) -> p x", p=P),
                )
                h_bigs_b[b].append(hb)

            issue_full(0, 0)
            issue_full(0, 1)
            # combined-batch packed tails: ONE [128, 2*CT] tile per layer
            # (partition p: batch 0's 96 tail values then batch 1's), 12
            # DMAs instead of 24 and a single 11-add chain that runs
            # entirely at the stream head.
            h_tails = []
            if T:
                for l in range(L):
                    htl = ht_pool.tile(
                        [P, 2 * CT], F32, tag="ht", name=f"ht_{l}"
                    )
                    tail_eng = nc.sync if l % 2 == 0 else nc.scalar
                    tail_eng.dma_start(
                        out=htl.rearrange("p (b x) -> p b x", b=BPC),
                        in_=hs[l, :, SFULL * H : SP * H].rearrange(
                            "b (p x) -> p b x", p=P
                        ),
                    )
                    h_tails.append(htl)
            NFULL = L - 1  # layers l0..l10 stream whole; only l11 as pieces
            c11_b = [None, None]
            for l in range(2, NFULL):
                issue_full(0, l)
            # b1's first two layers BEFORE b0's last-layer pieces: its
            # DVE chain can start ~3us earlier; b0's piece matmuls have
            # tens of us of slack before b1's drain needs PSUM.
            issue_full(1, 0)
            issue_full(1, 1)
            for b in range(BPC):
                evens, odds = engs[b]
                if b > 0:
                    for l in range(2, NFULL):
                        issue_full(b, l)
                # last layer as (j, n0) pieces; j0/j2 on `odds` (which
                # carried 5 fulls), j1 on `evens` (6 fulls).  (Putting
                # ALL pieces on `odds` to equalize bytes was tried and
                # regressed 8us: it skews the per-batch segment sizes
                # against the early-issued next-batch fulls.)
                c11 = {}
                l11_ap = hs[L - 1, b, 0 : SFULL * H].rearrange("(p x) -> p x", p=P)
                for j in range(3):
                    ch_eng = evens if j == 1 else odds
                    for n0, nl in NCHUNKS:
                        cb = c_pool.tile(
                            [P, nl], F32, tag=f"c{j}n{n0}", name=f"c_{b}_{j}_{n0}"
                        )
                        ch_eng.dma_start(
                            out=cb, in_=l11_ap[:, j * H + n0 : j * H + n0 + nl]
                        )
                        c11[(j, n0)] = cb
                c11_b[b] = c11

            # =========== phase 3a: the combined tail chain (all inputs
            # land at the stream head), both DRAM bounces, then the
            # embedding gathers (needed only by the stores) ===========
            tail16_b = [None, None]
            if T:
                hsumt = hsum_pool.tile([P, 2 * CT], F32, tag="hsumt")
                nc.vector.tensor_tensor(
                    hsumt, h_tails[0], h_tails[1], op=mybir.AluOpType.add
                )
                for l in range(2, L):
                    nc.vector.tensor_tensor(
                        hsumt, hsumt, h_tails[l], op=mybir.AluOpType.add
                    )
                for b in range(BPC):
                    nc.gpsimd.dma_start(
                        out=tsc[b, :].rearrange("(p x) -> p x", p=P),
                        in_=hsumt[:, b * CT : (b + 1) * CT],
                    )
                    tail16 = tail_pool.tile(
                        [T, H], F32, tag="t16", name=f"t16_{b}"
                    )
                    nc.gpsimd.dma_start(
                        out=tail16, in_=tsc[b, :].rearrange("(t x) -> t x", t=T)
                    )
                    tail16_b[b] = tail16
            obufs_b = []
            for b in range(BPC):
                obufs = []
                for wt in range(WT):
                    obuf = o_pool.tile(
                        [P, E + H], BF16, tag="obuf", name=f"obuf_{b}_{wt}"
                    )
                    nc.gpsimd.indirect_dma_start(
                        out=obuf[:, 0:E],
                        out_offset=None,
                        in_=emb[:, :],
                        in_offset=bass.IndirectOffsetOnAxis(
                            ap=wi_i[:, b * WT + wt : b * WT + wt + 1], axis=0
                        ),
                    )
                    obufs.append(obuf)
                obufs_b.append(obufs)

            # =========== phase 3b: per-batch layer sums + span matmuls =
            b0_stores = []
            for b in range(BPC):
                h_bigs, c11, hsum = h_bigs_b[b], c11_b[b], hsums_b[b]
                maskT_full, maskT_tail = maskT_fulls[b], maskT_tails[b]
                tail16, obufs = tail16_b[b], obufs_b[b]
                # exact f32 sum of layers l0..l10 on the DVE
                nc.vector.tensor_tensor(
                    hsum, h_bigs[0][:, 0:CF], h_bigs[1][:, 0:CF],
                    op=mybir.AluOpType.add,
                )
                for l in range(2, NFULL):
                    nc.vector.tensor_tensor(
                        hsum, hsum, h_bigs[l][:, 0:CF],
                        op=mybir.AluOpType.add,
                    )
                # span matmuls: the tail matmul OPENS each PSUM zone
                # (its data has been ready since ~25us); then per (j, n0)
                # piece: the DVE add finalizes hsum[:, jH+n0 : jH+n0+nl]
                # and unlocks exactly the two matmuls that consume it.
                pouts = []
                for wt in range(WT):
                    pout = pout_pool.tile(
                        [P, H], F32, space="PSUM", tag=f"pout{wt}",
                        name=f"pout{wt}_{b}",
                    )
                    pouts.append(pout)
                if T:
                    for wt in range(WT):
                        for n0, nl in NCHUNKS:
                            nc.tensor.matmul(
                                pouts[wt][:, n0 : n0 + nl],
                                lhsT=maskT_tail[:, wt * P : (wt + 1) * P],
                                rhs=tail16[:, n0 : n0 + nl],
                                start=True,
                                stop=False,
                            )
                for j in range(3):
                    for n0, nl in NCHUNKS:
                        nc.vector.tensor_tensor(
                            hsum[:, j * H + n0 : j * H + n0 + nl],
                            hsum[:, j * H + n0 : j * H + n0 + nl],
                            c11[(j, n0)],
                            op=mybir.AluOpType.add,
                        )
                        for wt in range(WT):
                            col = (j * WT + wt) * P
                            nc.tensor.matmul(
                                pouts[wt][:, n0 : n0 + nl],
                                lhsT=maskT_full[:, col : col + P],
                                rhs=hsum[:, j * H + n0 : j * H + n0 + nl],
                                start=(j == 0 and not T),
                                stop=(j == 2),
                            )
                # PSUM -> bf16 row tiles.  Last batch: wt0 on ACT in
                # parallel with wt1 on the (by then idle) DVE; earlier
                # batches both on ACT (a DVE copy would stall the
                # in-order DVE stream on this batch's matmuls).
                for n0, nl in NCHUNKS:
                    nc.scalar.copy(
                        obufs[0][:, E + n0 : E + n0 + nl],
                        pouts[0][:, n0 : n0 + nl],
                    )
                if b < BPC - 1:
                    for n0, nl in NCHUNKS:
                        nc.scalar.copy(
                            obufs[1][:, E + n0 : E + n0 + nl],
                            pouts[1][:, n0 : n0 + nl],
                        )
                    # stores ride SWDGE but are EMITTED after the whole
                    # b1 block so the gpsimd engine's in-order stream
                    # isn't parked waiting on b0's obufs.
                    b0_stores.append((b, obufs))
                else:
                    nc.vector.tensor_copy(obufs[1][:, E : E + H], pouts[1])
                    nc.scalar.dma_start(out=out[b, 0:P, :], in_=obufs[0])
                    nc.sync.dma_start(out=out[b, P : 2 * P, :], in_=obufs[1])
            for b, obufs in b0_stores:
                for wt in range(WT):
                    nc.gpsimd.dma_start(
                        out=out[b, wt * P : (wt + 1) * P, :], in_=obufs[wt]
                    )

    nc.compile()
    return nc


_NC = {}


def _tail_for(s_used):
    """Round the needed tail (beyond 384) up to a power of two <= 128."""
    if s_used <= SFULL:
        return 0
    t = s_used - SFULL
    p = 1
    while p < t:
        p *= 2
    return min(p, P)


def _get_program(T=16):
    if T not in _NC:
        _NC[T] = build_program(T)
    return _NC[T]


def _idx_stage(a):
    """[BPC, W] int -> [P, BPC*WT] with col (b*WT+wt) = word-tile wt of b."""
    a = np.asarray(a).astype(np.int32).reshape(BPC, WT, P)
    return np.ascontiguousarray(a.transpose(2, 0, 1).reshape(P, BPC * WT))


def make_in_maps(word_indices, span_start, span_end, W_embed, hidden_states, T):
    import ml_dtypes

    SP = SFULL + T
    emb = np.ascontiguousarray(np.asarray(W_embed).astype(ml_dtypes.bfloat16))
    in_maps = []
    for c in range(NCORES):
        bsl = slice(BPC * c, BPC * (c + 1))
        hsc = np.ascontiguousarray(
            hidden_states[:, bsl, :SP, :], dtype=np.float32
        ).reshape(L, BPC, SP * H)
        in_maps.append(
            {
                "word_indices": _idx_stage(word_indices[bsl]),
                "span_start": _idx_stage(span_start[bsl]),
                "span_end": _idx_stage(span_end[bsl]),
                "W_embed": emb,
                "hidden_states": hsc,
            }
        )
    return in_maps


def run(word_indices, span_start, span_end, W_embed, hidden_states, **run_kwargs):
    from concourse.bass_utils import run_bass_kernel_spmd

    s_used = int(np.max(np.asarray(span_end)[:, -1]))
    T = _tail_for(s_used)
    nc = _get_program(T)
    in_maps = make_in_maps(
        word_indices, span_start, span_end, W_embed, hidden_states, T
    )
    res = run_bass_kernel_spmd(nc, in_maps, core_ids=list(range(NCORES)), **run_kwargs)
    out = np.concatenate(
        [np.asarray(res.results[c]["out"], dtype=np.float32) for c in range(NCORES)],
        axis=0,
    )
    return out, res


def kernel(word_indices, span_start, span_end, W_embed, hidden_states):
    out, _ = run(word_indices, span_start, span_end, W_embed, hidden_states)
    return out
